# revision 5
# baseline (speedup 1.0000x reference)
"""ConvPMF Trainium2 kernel: 8-core data-parallel over the doc axis.

Per core (16 docs = 256 seqs of 256 tokens):
  - batched indirect DMA gathers bf16 embedding rows -> SBUF [token, E]
  - SBUF-source dma_gather(transpose=True) flips to [E, token] (16-bit
    granularity transpose; identity int16 index table)
  - conv1d(SAME, W=5) as 5 column-shifted PSUM-accumulated matmuls,
    col-tiled 4 seqs per PSUM bank (M=32 each)
  - tanh on ACT (PSUM->SBUF), fused max/exp-sum/entropy-sum reductions
  - ratings + entropy epilogue on-chip; host sums 8 partial outputs
"""

import numpy as np
import ml_dtypes

import concourse.bass as bass
import concourse.mybir as mybir
import concourse.tile as tile
from concourse.masks import make_identity
from concourse.bass_utils import run_bass_kernel_spmd

B, R, L = 128, 16, 256
NU, F, V, E, W = 100000, 32, 50000, 128, 5
NC = 8
B_CORE = B // NC            # 16 docs per core
S_CORE = B_CORE * R         # 256 seqs per core
GROUPS = S_CORE // 4        # 64 groups of 4 seqs
BATCH_G = 8                 # groups per DMA batch
NBATCH = GROUPS // BATCH_G  # 8
TOK_BATCH = BATCH_G * 4 * L  # 8192 tokens per batch
F32 = mybir.dt.float32
BF16 = mybir.dt.bfloat16
I32 = mybir.dt.int32
I16 = mybir.dt.int16

# conv shift ranges: fm[:, o0:o1] += W_w.T @ x[:, r0:r1]   (l_out = l_in - w + 2)
_SHIFT = {0: (2, 256, 0, 254), 1: (1, 256, 0, 255), 2: (0, 256, 0, 256),
          3: (0, 255, 1, 256), 4: (0, 254, 2, 256)}


def _fix_tail_drain(nc, max_waits=1):
    """this walrus build only accepts 1 sync-wait per instruction on several
    ISA structs; hoist extras onto preceding single-wait NoOps."""
    for fn in nc.m.functions:
        for bb in fn.blocks:
            changed, new_list = False, []
            for inst in bb.instructions:
                if (inst.sync_info is not None
                        and len(inst.sync_info.on_wait) > max_waits):
                    waits = list(inst.sync_info.on_wait)
                    for k, w in enumerate(waits[:-max_waits]):
                        d = mybir.InstNoOp(name=f"{inst.name}-w{k}", ins=[], outs=[])
                        d.engine = inst.engine
                        d.sync_info = mybir.SyncInfo(on_wait=[w], on_update=[])
                        new_list.append(d)
                    inst.sync_info.on_wait = waits[-max_waits:]
                    changed = True
                new_list.append(inst)
            if changed:
                bb.instructions = new_list


def _build_program():
    nc = bass.Bass()
    AF = mybir.ActivationFunctionType
    ALU = mybir.AluOpType
    AX = mybir.AxisListType

    table = nc.declare_dram_parameter("table", [V, E], F32, isOutput=False)
    wuser = nc.declare_dram_parameter("wuser", [NU, F], F32, isOutput=False)
    offs = nc.declare_dram_parameter("offs", [128, 512], I32, isOutput=False)
    sel = nc.declare_dram_parameter("sel", [128, F], F32, isOutput=False)
    wall = nc.declare_dram_parameter("wall", [128, W * F], F32, isOutput=False)
    uidx = nc.declare_dram_parameter("uidx", [B_CORE, 1], I32, isOutput=False)
    ratings = nc.declare_dram_parameter("ratings", [B_CORE, 1], F32, isOutput=True)
    ent = nc.declare_dram_parameter("ent", [1, 1], F32, isOutput=True)

    with tile.TileContext(nc) as tc:
        with (
            tc.tile_pool(name="const", bufs=1) as cpool,
            tc.tile_pool(name="xg", bufs=4) as xgpool,
            tc.tile_pool(name="tp", bufs=2, space="PSUM") as tppool,
            tc.tile_pool(name="xt", bufs=3) as xtpool,
            tc.tile_pool(name="fmp", bufs=4, space="PSUM") as fmpool,
            tc.tile_pool(name="fms", bufs=3) as fmspool,
            tc.tile_pool(name="q", bufs=3) as qpool,
            tc.tile_pool(name="scr", bufs=3) as scrpool,
            tc.tile_pool(name="ep", bufs=1) as eppool,
            tc.tile_pool(name="epp", bufs=1, space="PSUM") as epppool,
        ):
            offs_sb = cpool.tile([128, 512], I32)
            nc.sync.dma_start(offs_sb[:], offs[:, :])
            sel_sb = cpool.tile([128, F], F32)
            nc.sync.dma_start(sel_sb[:], sel[:, :])
            w_sb = cpool.tile([128, W * F], F32)
            nc.sync.dma_start(w_sb[:], wall[:, :])
            uidx_sb = cpool.tile([B_CORE, 1], I32)
            nc.sync.dma_start(uidx_sb[:], uidx[:, :])
            ones_sb = cpool.tile([128, 1], F32)
            nc.vector.memset(ones_sb[:], 1.0)
            ident = cpool.tile([128, 128], F32)
            make_identity(nc, ident[:])

            m_acc = cpool.tile([128, GROUPS], F32)   # -max per row, per group
            s_acc = cpool.tile([128, GROUPS], F32)   # sum exp
            u_acc = cpool.tile([128, GROUPS], F32)   # sum q*(x-m)

            for b in range(NBATCH):
                for gl in range(BATCH_G):
                    g = b * BATCH_G + gl
                    xt = xtpool.tile([128, 1024], F32)
                    for sc in range(8):  # (seq-in-group, half) chunks
                        xg = xgpool.tile([128, 128], F32)
                        nc.gpsimd.indirect_dma_start(
                            out=xg[:], out_offset=None, in_=table[:, :],
                            in_offset=bass.IndirectOffsetOnAxis(
                                ap=offs_sb[:, g * 8 + sc:g * 8 + sc + 1], axis=0))
                        tp = tppool.tile([128, 128], F32)
                        nc.tensor.transpose(tp[:], xg[:], ident[:])
                        s, cc = sc // 2, sc % 2
                        nc.vector.tensor_copy(
                            xt[:, s * 256 + cc * 128:s * 256 + cc * 128 + 128], tp[:])
                    fm = fmpool.tile([128, 256], F32)
                    for wi in (2, 0, 1, 3, 4):
                        o0, o1, r0, r1 = _SHIFT[wi]
                        for j in range(4):
                            base = j * L
                            nc.tensor.matmul(
                                fm[32 * j:32 * j + 32, o0:o1],
                                lhsT=w_sb[:, wi * F:(wi + 1) * F],
                                rhs=xt[:, base + r0:base + r1],
                                start=(wi == 2),
                                stop=(wi == 4),
                                tile_position=(0, 32 * j),
                                skip_group_check=True,
                            )
                    fms = fmspool.tile([128, 256], F32)
                    nc.scalar.activation(fms[:], fm[:], AF.Tanh)
                    nc.vector.tensor_reduce(
                        m_acc[:, g:g + 1], fms[:], axis=AX.X, op=ALU.max, negate=True)
                    q = qpool.tile([128, 256], F32)
                    nc.scalar.activation(
                        q[:], fms[:], AF.Exp, bias=m_acc[:, g:g + 1], scale=1.0,
                        accum_out=s_acc[:, g:g + 1])
                    scr = scrpool.tile([128, 256], F32)
                    nc.vector.scalar_tensor_tensor(
                        out=scr[:], in0=fms[:], scalar=m_acc[:, g:g + 1], in1=q[:],
                        op0=ALU.add, op1=ALU.mult, accum_out=u_acc[:, g:g + 1])

            # ---- epilogue: ratings ----
            dacc = eppool.tile([128, B_CORE], F32)
            nc.vector.tensor_reduce(
                dacc[:], m_acc[:].rearrange("p (d a) -> p d a", a=4),
                axis=AX.X, op=ALU.add)
            it_ps = epppool.tile([B_CORE, F], F32)
            nc.tensor.matmul(it_ps[:], lhsT=dacc[:], rhs=sel_sb[:],
                             start=True, stop=True)
            us = eppool.tile([B_CORE, F], F32)
            nc.gpsimd.indirect_dma_start(
                out=us[:], out_offset=None, in_=wuser[:, :],
                in_offset=bass.IndirectOffsetOnAxis(ap=uidx_sb[:, :1], axis=0))
            rt_scr = eppool.tile([B_CORE, F], F32)
            rt = eppool.tile([B_CORE, 1], F32)
            nc.vector.scalar_tensor_tensor(
                out=rt_scr[:], in0=it_ps[:], scalar=1.0, in1=us[:],
                op0=ALU.mult, op1=ALU.mult, accum_out=rt[:])
            nc.sync.dma_start(ratings[:, :], rt[:])

            # ---- epilogue: entropy ----
            rs = eppool.tile([128, GROUPS], F32)
            nc.vector.reciprocal(rs[:], s_acc[:])
            logs = eppool.tile([128, GROUPS], F32)
            nc.scalar.activation(logs[:], s_acc[:], AF.Ln)
            t1 = eppool.tile([128, GROUPS], F32)
            nc.vector.tensor_tensor(out=t1[:], in0=u_acc[:], in1=rs[:], op=ALU.mult)
            er = eppool.tile([128, GROUPS], F32)
            nc.vector.tensor_tensor(out=er[:], in0=logs[:], in1=t1[:], op=ALU.subtract)
            ec = eppool.tile([128, 1], F32)
            nc.vector.tensor_reduce(ec[:], er[:], axis=AX.X, op=ALU.add)
            ent_ps = epppool.tile([1, 1], F32)
            nc.tensor.matmul(ent_ps[:], lhsT=ec[:], rhs=ones_sb[:],
                             start=True, stop=True)
            ent_sb = eppool.tile([1, 1], F32)
            nc.scalar.activation(ent_sb[:], ent_ps[:], AF.Copy)
            nc.sync.dma_start(ent[:, :], ent_sb[:])

    _fix_tail_drain(nc)
    return nc


_NC_CACHE = None


def _get_program():
    global _NC_CACHE
    if _NC_CACHE is None:
        _NC_CACHE = _build_program()
    return _NC_CACHE


def _host_prep(user_indices, docs, w_user, embed_table, conv_w):
    table_bf = np.ascontiguousarray(embed_table.astype(np.float32))
    # lhsT layout: wall[e, w*F + f] = conv_w[f, e, w]
    wall = np.ascontiguousarray(
        conv_w.transpose(1, 2, 0).reshape(E, W * F).astype(np.float32))
    sel = np.zeros((128, F), np.float32)
    p = np.arange(128)
    sel[p, p % F] = -1.0 / R
    wuser = np.ascontiguousarray(w_user.astype(np.float32))

    in_maps = []
    docs_i = docs.reshape(B, R, L).astype(np.int32)
    uid = user_indices.astype(np.int32)
    for c in range(NC):
        tok = docs_i[c * B_CORE:(c + 1) * B_CORE].reshape(S_CORE, L)
        # offs[p, g*8 + s*2 + cc] = tok[g*4+s, cc*128+p]
        t = tok.reshape(GROUPS, 4, 2, 128)
        offs = np.ascontiguousarray(
            t.transpose(3, 0, 1, 2).reshape(128, 512)).astype(np.int32)
        in_maps.append({
            "table": table_bf,
            "wuser": wuser,
            "offs": offs,
            "sel": sel,
            "wall": wall,
            "uidx": np.ascontiguousarray(
                uid[c * B_CORE:(c + 1) * B_CORE].reshape(B_CORE, 1)),
        })
    return in_maps


def kernel(user_indices, docs, w_user, embed_table, conv_w, bias):
    user_indices = np.asarray(user_indices)
    docs = np.asarray(docs)
    w_user = np.asarray(w_user, dtype=np.float32)
    embed_table = np.asarray(embed_table, dtype=np.float32)
    conv_w = np.asarray(conv_w, dtype=np.float32)
    bias = np.asarray(bias, dtype=np.float32)

    nc = _get_program()
    in_maps = _host_prep(user_indices, docs, w_user, embed_table, conv_w)
    res = run_bass_kernel_spmd(nc, in_maps, core_ids=list(range(NC)))
    ratings = np.concatenate(
        [res.results[c]["ratings"].reshape(B_CORE) for c in range(NC)])
    ratings = (ratings + bias[0]).astype(np.float32)
    ent_sum = np.float32(sum(float(res.results[c]["ent"][0, 0]) for c in range(NC)))
    entropy = np.float32(ent_sum / (B * R * F))
    return ratings, entropy
